# revision 52
# baseline (speedup 1.0000x reference)
"""BKT forward kernel for Trainium2 (8 NeuronCores, data-parallel over batch).

Math: in odds space rho = L/(1-L) the BKT update is affine:
    rho' = a_t * rho + lam,   a_t = y ? (1-s)/(g(1-l)) : s/((1-g)(1-l)),
    lam = l/(1-l),
because the per-step Mobius map fixes L=1. The clip L <= 1-EPS becomes
rho <= R. Pin steps (where the clip binds) are detected with a
multiplicative tracker v_t = min(a_t * v_{t-1}, 1) (v = product part of
rho/R; the lam/R ~ 1e-5 shift is below output precision), so the mask is
just (v < 1) and no logarithm of a is needed. The trajectory is
reconstructed with a mult/add scan whose operands are masked to force a
reset at pins. The scan runs in units of S = 2^-60 (host prescales the
additive operands) so the worst-case missed-pin overshoot stays far below
both f32 inf and the scalar engine's Ln range limit of 2^64.

Output tail uses one activation table (natural_log_exp_and_others):
    w = Ln(S*p + S) = ln(S*(1+rho));  r = Exp(-w + ln S) = 1/(1+rho)
    lat = 1 - r (Act Identity);  cor = (1-s) - (1-s-g)*r  (Pool TS)
so there are zero activation-table reloads after the first.

Engine legality on real V3 cores (probed): tensor_tensor_scan is DVE-only
(Pool rejects the opcode); Pool handles tensor_scalar / tensor_tensor /
copy / memset but not scalar_tensor_tensor. Placement per 128-student
group (TRN2 cost model, ns):
  vector(DVE):  v-scan 594 (per group), p-scan 549 (merged across the 4
                groups of a jumbo via d0=0/d1=rho0 boundary columns),
                notm 148 (jumbo 4x), d1 194 (bf16 4x, partially on Pool)
  gpsimd(Pool): a 427, d0=a*notm 427 (jumbo TT), cor 427
  scalar(Act):  Ln 474 + Exp 474 + lat 474 (jumbo over 4 groups)
Outputs are written fp16 (halves DMA traffic; rel-err impact < 1e-4).

Validated-but-unimplemented next step (numpy, this dataset): pair-decimated
pin detection. Track v2' = min(a2*v2, 1) over step PAIRS (a2 = a_2t*a_2t+1,
256-long scans), mask the even-chain composite update
rho' = m2*(a2*rho + lam*(1+a_odd)) + (1-m2)*R, and reconstruct odd steps
elementwise as rho_odd = min(a_even*rho + lam, R). Measured global L2
7.88e-3 vs 8.37e-3 shipped (the odd-step clamp is exact where the current
scheme approximates). Halves v-scan DVE time; nets ~-2.4us after
rebalancing (a2 product op offsets part of the saving). Do NOT instead
merge v-scan windows across groups: any chained window >= 1024 steps
overflows f32 via zero-stick (measured).
"""

import numpy as np

B_FULL = 65536
T = 512
N_CORES = 8
B_CORE = B_FULL // N_CORES          # 8192
NG = B_CORE // 128                  # 64 groups of 128 students
JG = 4                              # groups per jumbo iteration
NJ = NG // JG                       # 16 uniform jumbos (layout reference)
NJB = 17                            # actual jumbo count
# ramped start: small first jumbos deliver p32s to the scalar engine
# early (its first op otherwise starves ~11us behind the first p-scan)
JUMBOS = [(0, 1), (1, 3)] + [(4 + m * 4, 4) for m in range(15)]
LAT_DVE_JUMBOS = frozenset({14, 15, 16})
D1_ACT_JUMBOS = frozenset()
NP = 8                              # param slots per group
NC_EXTRA = 4                        # const cols: SCALE, ln(S), 1.0, R*S
EPS = 1e-6
SCALE = 2.0 ** -60                  # p-scan unit

_cache = {}


def _R():
    Lstar = np.float32(1.0) - np.float32(EPS)
    return float(np.float64(Lstar) / (1.0 - np.float64(Lstar)))


def _make_bacc_cls():
    """Bacc subclass whose activation-table pass resolves Ln and Exp to the
    single canonical set that contains both (`natural_log_exp_and_others`,
    id 6). The default pass first-matches them to two different sets and
    inserts a 1.3us table reload before every activation. We run the same
    rust pass but hide Ln/Exp from the earlier sets; list order (and hence
    the emitted canonical act_func_set_id) is unchanged."""
    import bass_rust as _bass_rust
    import concourse.bacc as bacc
    import concourse.mybir as mybir
    from concourse.hw_specs import get_activation_tables

    class _Bacc(bacc.Bacc):
        def insert_act_table_loads(self):
            has_activation = any(
                isinstance(i, mybir.InstActivation)
                for b in self.main_func.blocks
                for i in b.instructions
            )
            if not has_activation:
                return
            lnexp = {
                mybir.ActivationFunctionType.Ln,
                mybir.ActivationFunctionType.Exp,
            }
            tables = []
            for name, funcs in get_activation_tables(self.m.arch).items():
                if name != "natural_log_exp_and_others":
                    funcs = set(funcs) - lnexp
                tables.append((name, funcs))
            _bass_rust.insert_act_table_loads(self, tables)

    return _Bacc


def _build_bass(d1_pool_jumbos=frozenset()):
    """d1_pool_jumbos: jumbo indices whose d1 TS runs on Pool instead of
    DVE (load-balancing knob)."""
    import concourse.mybir as mybir
    from concourse.tile import TileContext

    dt = mybir.dt
    op = mybir.AluOpType
    act = mybir.ActivationFunctionType

    nc = _make_bacc_cls()(None, target_bir_lowering=False)
    y_d = nc.dram_tensor("y", [NJB * 128, JG * T], dt.int8, kind="ExternalInput")
    par_d = nc.dram_tensor(
        "par", [128, NG * NP + NC_EXTRA], dt.float32, kind="ExternalInput"
    )
    lat_d = nc.dram_tensor("lat", [NJB * 128, JG * T], dt.float16, kind="ExternalOutput")
    cor_d = nc.dram_tensor("cor", [NJB * 128, JG * T], dt.float16, kind="ExternalOutput")

    R = _R()
    NPAR = NG * NP
    jumbos = JUMBOS

    with TileContext(nc) as tc:
        with (
            tc.tile_pool(name="const", bufs=1) as cpool,
            tc.tile_pool(name="work", bufs=3) as pool,
            tc.tile_pool(name="unit", bufs=2) as upool,
            tc.tile_pool(name="big", bufs=2) as bigpool,
            tc.tile_pool(name="wpool", bufs=1) as wpool,
        ):
            ones16 = cpool.tile([128, T], dt.bfloat16)
            nc.gpsimd.memset(ones16[:], 1.0)
            par_t = cpool.tile([128, NPAR + NC_EXTRA], dt.float32)
            nc.sync.dma_start(par_t[:, 0:32], par_d[:, 0:32])
            nc.sync.dma_start(par_t[:, 32:], par_d[:, 32:])
            # per-engine copies so scalar-AP reads are same-engine deps
            par_dv = cpool.tile([128, NPAR + NC_EXTRA], dt.float32)
            nc.vector.tensor_copy(par_dv[:, 0:32], par_t[:, 0:32])
            nc.vector.tensor_copy(par_dv[:, 32:], par_t[:, 32:])
            par_gp = cpool.tile([128, NPAR + NC_EXTRA], dt.float32)
            nc.gpsimd.tensor_copy(par_gp[:, 0:32], par_t[:, 0:32])
            nc.gpsimd.tensor_copy(par_gp[:, 32:], par_t[:, 32:])
            par_ac = cpool.tile([128, NPAR + NC_EXTRA], dt.float32)
            nc.scalar.copy(par_ac[:, 0:32], par_t[:, 0:32])
            nc.scalar.copy(par_ac[:, 32:], par_t[:, 32:])
            unit_state = {}

            for j, (G, k) in enumerate(jumbos):
                FD = k * T
                BD = k * (T + 1) - 1
                PD = k * (T + 1)
                r0, r1 = j * 128, (j + 1) * 128
                y8 = pool.tile([128, FD], dt.int8, tag="y8")
                nc.sync.dma_start(y8[:], y_d[r0:r1, 0:FD])

                # a in boundary layout [128, PD] (data at 513g..513g+511)
                # so d0/d1/notm share one indexing scheme; v-scans stay
                # per-group: a multiplicative min-scan cannot revive a state
                # that flushed to zero in a deep dip, so chaining groups
                # would let the zero stick across resets and the unmasked
                # p-recurrence overflow (observed). Per-group initial APs
                # contain any flush to one 512-step window.
                a32 = pool.tile([128, PD], dt.float32, tag="a32")
                v16 = pool.tile([128, PD], dt.bfloat16, tag="v16")
                b0 = G * NP
                for gidx in range(k):
                    jj = G + gidx
                    b = jj * NP
                    gs = slice(gidx * T, (gidx + 1) * T)
                    bs = slice(gidx * (T + 1), gidx * (T + 1) + T)
                    if j == 0:
                        # first chain: keep a on DVE so the opening v-scan
                        # needs no cross-engine hop while DVE is idle anyway
                        nc.vector.tensor_scalar(
                            a32[:, bs], y8[:, gs],
                            par_dv[:, b + 1 : b + 2], par_dv[:, b + 0 : b + 1],
                            op.mult, op.add,
                        )
                    else:
                        nc.gpsimd.tensor_scalar(
                            a32[:, bs], y8[:, gs],
                            par_gp[:, b + 1 : b + 2], par_gp[:, b + 0 : b + 1],
                            op.mult, op.add,
                        )
                    nc.vector.tensor_tensor_scan(
                        v16[:, bs], a32[:, bs], ones16[:],
                        par_dv[:, b + 5 : b + 6], op.mult, op.min,
                    )

                notm = pool.tile([128, PD], dt.float16, tag="notm")
                nc.vector.tensor_scalar(
                    notm[:].rearrange("p (g t) -> p g t", g=k, t=T + 1)[
                        :, :, 0:T
                    ],
                    v16[:].rearrange("p (g t) -> p g t", g=k, t=T + 1)[
                        :, :, 0:T
                    ],
                    1.0, None, op.is_lt,
                )

                # d1/d0 in boundary layout [128, 2051]: group g data at
                # cols 513g..513g+511, boundary col 513g+512 between groups
                # resets the merged p-scan (d0=0 -> state := d1 = rho0*S).
                d1bf = pool.tile([128, PD], dt.bfloat16, tag="d1bf")
                d0_16 = pool.tile([128, PD], dt.float16, tag="d0")
                for gidx in range(k):
                    jj = G + gidx
                    b = jj * NP
                    gs = slice(gidx * T, (gidx + 1) * T)
                    bs = slice(gidx * (T + 1), gidx * (T + 1) + T)
                    # d1 = notm*(lam-R)*S + R*S  (scan runs in S-units);
                    # group 0 on Pool, group 1 on Act for mid-stream jumbos
                    # (both engines have slack vs the DVE pole)
                    if gidx == 0:
                        nc.gpsimd.tensor_scalar(
                            d1bf[:, bs], notm[:, bs],
                            par_gp[:, b + 2 : b + 3], R * SCALE,
                            op.mult, op.add,
                        )
                    elif gidx == 1 and j in D1_ACT_JUMBOS:
                        nc.scalar.activation(
                            d1bf[:, bs], notm[:, bs], act.Identity,
                            bias=par_ac[:, NPAR + 3 : NPAR + 4],
                            scale=par_ac[:, b + 2 : b + 3],
                        )
                    else:
                        nc.vector.tensor_scalar(
                            d1bf[:, bs], notm[:, bs],
                            par_dv[:, b + 2 : b + 3], R * SCALE,
                            op.mult, op.add,
                        )
                d0_dst = d0_16[:].rearrange("p (g t) -> p g t", g=k, t=T + 1)[
                    :, :, 0:T
                ]
                a_src = a32[:].rearrange("p (g t) -> p g t", g=k, t=T + 1)[
                    :, :, 0:T
                ]
                m_src = notm[:].rearrange("p (g t) -> p g t", g=k, t=T + 1)[
                    :, :, 0:T
                ]
                nc.gpsimd.tensor_tensor(d0_dst, a_src, m_src, op.mult)
                for gidx in range(k - 1):
                    bcol = gidx * (T + 1) + T
                    jj2 = G + gidx + 1
                    nc.gpsimd.memset(d0_16[:, bcol : bcol + 1], 0.0)
                    nc.gpsimd.tensor_copy(
                        d1bf[:, bcol : bcol + 1],
                        par_gp[:, jj2 * NP + 6 : jj2 * NP + 7],
                    )

                NU = 1
                p32u = bigpool.tile([128, PD], dt.float32, tag="p32")
                nc.gpsimd.tensor_copy(p32u[:, 0:1], par_gp[:, b0 + 6 : b0 + 7])
                nc.vector.tensor_tensor_scan(
                    p32u[:, 1:PD], d0_16[:, 0:BD], d1bf[:, 0:BD],
                    par_dv[:, b0 + 6 : b0 + 7], op.mult, op.add,
                )

                # w = ln(p_scaled + S); r = exp(-w + ln S) = 1/(1+rho)
                UD = PD
                NGU = k
                G0 = G
                w32 = wpool.tile([128, UD], dt.float32, tag="w32")
                nc.scalar.activation(
                    w32[:], p32u[:], act.Ln,
                    bias=par_ac[:, NPAR : NPAR + 1],
                )
                r16 = upool.tile([128, UD], dt.float16, tag="r16")
                nc.scalar.activation(
                    r16[:], w32[:], act.Exp,
                    bias=par_ac[:, NPAR + 1 : NPAR + 2], scale=-1.0,
                )

                lat16 = upool.tile([128, UD], dt.float16, tag="lat16")
                nc.scalar.activation(
                    lat16[:], r16[:], act.Identity,
                    bias=par_ac[:, NPAR + 2 : NPAR + 3], scale=-1.0,
                )
                cor16 = upool.tile([128, UD], dt.float16, tag="cor16")
                for gu in range(NGU):
                    jj = G0 + gu
                    b = jj * NP
                    ps = slice(gu * (T + 1), gu * (T + 1) + T)
                    nc.gpsimd.tensor_scalar(
                        cor16[:, ps], r16[:, ps],
                        par_gp[:, b + 3 : b + 4], par_gp[:, b + 4 : b + 5],
                        op.mult, op.add,
                    )

                lat_src = lat16[:].rearrange("p (g t) -> p g t", g=NGU)[:, :, 0:T]
                cor_src = cor16[:].rearrange("p (g t) -> p g t", g=NGU)[:, :, 0:T]
                nc.sync.dma_start(lat_d[r0:r1, 0:FD], lat_src)
                nc.sync.dma_start(cor_d[r0:r1, 0:FD], cor_src)
    nc.compile()
    return nc


def _host_params(X, learn_w, guess_w, slip_w, prior_w):
    f32 = np.float32
    f64 = np.float64

    def sig(w):
        return 1.0 / (1.0 + np.exp(-w.astype(f64)))

    l = sig(learn_w[X[:, 0], 0])
    g = sig(guess_w[X[:, 1], 0])
    s = sig(slip_w[X[:, 2], 0])
    p = sig(prior_w[X[:, 3], 0])
    R = f64(_R())
    S = f64(SCALE)
    a1 = (1.0 - s) / (g * (1.0 - l))
    a0 = s / ((1.0 - g) * (1.0 - l))
    lam = l / (1.0 - l)
    rho0 = p / (1.0 - p)
    par = np.stack(
        [
            a0,                  # 0
            a1 - a0,             # 1  d
            (lam - R) * S,       # 2  lamR (S-units)
            s + g - 1.0,         # 3  negcd  (cor = ghat + negcd*r)
            1.0 - s,             # 4  ghat
            rho0 / R,            # 5  v0
            rho0 * S,            # 6  rho0 (S-units)
            np.zeros_like(a0),   # 7  pad
        ],
        axis=1,
    ).astype(f32)
    # per-core layout (128, NG*NP): partition p, col jj*NP+k = student jj*128+p
    par = par.reshape(N_CORES, NG, 128, NP).transpose(0, 2, 1, 3)
    par = par.reshape(N_CORES, 128, NG * NP)
    consts = np.broadcast_to(
        np.array([SCALE, np.log(f64(SCALE)), 1.0, R * S], dtype=f32),
        (N_CORES, 128, NC_EXTRA),
    )
    return np.ascontiguousarray(np.concatenate([par, consts], axis=2), dtype=f32)


def _host_y(y):
    y8 = np.where(np.asarray(y) < 0, 0, np.asarray(y)).astype(np.int8)
    y8 = y8.reshape(N_CORES, NG, 128, T)
    out = np.zeros((N_CORES, NJB * 128, JG * T), np.int8)
    for j, (G, k) in enumerate(JUMBOS):
        blk = y8[:, G : G + k].transpose(0, 2, 1, 3).reshape(N_CORES, 128, k * T)
        out[:, j * 128 : (j + 1) * 128, 0 : k * T] = blk
    return out


def _host_unshuffle(out):
    # [cores][NJB*128, JG*T] fp16 -> [B, T] f32
    out = np.stack(out)
    res = np.empty((N_CORES, NG, 128, T), np.float32)
    for j, (G, k) in enumerate(JUMBOS):
        blk = out[:, j * 128 : (j + 1) * 128, 0 : k * T].astype(np.float32)
        res[:, G : G + k] = blk.reshape(N_CORES, 128, k, T).transpose(0, 2, 1, 3)
    return np.ascontiguousarray(res.reshape(B_FULL, T))


D1_POOL_JUMBOS = frozenset({2, 6, 10, 14})


def get_nc():
    if "nc" not in _cache:
        _cache["nc"] = _build_bass(D1_POOL_JUMBOS)
    return _cache["nc"]


def kernel(X, y, learn_w, guess_w, slip_w, prior_w, _trace=False):
    from concourse import bass_utils

    y8 = _host_y(y)
    par = _host_params(
        np.asarray(X),
        np.asarray(learn_w, np.float32),
        np.asarray(guess_w, np.float32),
        np.asarray(slip_w, np.float32),
        np.asarray(prior_w, np.float32),
    )

    nc = get_nc()

    in_maps = [{"y": y8[i], "par": par[i]} for i in range(N_CORES)]
    res = bass_utils.run_bass_kernel_spmd(
        nc, in_maps, core_ids=list(range(N_CORES)), trace=_trace
    )
    outs = res.results
    cor = _host_unshuffle([outs[i]["cor"] for i in range(N_CORES)])
    lat = _host_unshuffle([outs[i]["lat"] for i in range(N_CORES)])
    if _trace:
        _cache["last_exec_time_ns"] = res.exec_time_ns
    return cor, lat
